# revision 6
# baseline (speedup 1.0000x reference)
"""MoE ExpertRouter kernel for 8 Trainium2 NeuronCores.

Strategy (expert-parallel, per the sharding hint): the host computes the
gate (a 67 M-MAC matmul, 0.05% of total FLOPs), does the top-k routing and
softmax weights, and all-to-alls tokens to experts as the sharding step.
Core e holds expert e's FFN weights resident in SBUF and runs
  yT = wt * (relu(x @ w1 + b1) @ w2 + b2)^T
for the ~2k tokens routed to it, with float32r (FP22) matmuls at full PE
rate. The host scatter-adds the two weighted expert outputs per token.

Everything is laid out so no transposes happen on device:
  mm1: hT[f,c] = sum_k w1[k,f] * xT[k,c]   (lhsT = w1 slice, rhs = xT slice)
  mm2: yT[d,c] = sum_f w2[f,d] * hT[f,c]   (lhsT = w2 slice, rhs = hT tile)
The host supplies xT (tokens transposed) and receives yT.
"""

import sys

try:
    import concourse.bass as bass
except ImportError:  # pragma: no cover
    sys.path.insert(0, "/opt/trn_rl_repo")
    import concourse.bass as bass

import numpy as np
import bass_rust
import concourse.mybir as mybir
from concourse.tile import TileContext
from concourse.bass_utils import run_bass_kernel_spmd

P = 128
D_MODEL = 1024
D_FF = 2048
N_EXPERTS = 8
N_CORES = 8
KO = D_MODEL // P   # 8  k-tiles for mm1
FO = D_FF // P      # 16 f-tiles
DO = D_MODEL // P   # 8  d-tiles for mm2
CHUNK = 512
NEG_INF = -1e9

F32 = mybir.dt.float32
F32R = mybir.dt.float32r

_nc_cache = {}


def _split_multiwait(nc):
    """The walrus in this env allows a single sync-wait per instruction;
    Tile's tail drain carries several. Hoist extras onto single-wait NOPs
    inserted immediately before the offending instruction."""
    k = 0
    for f in nc.m.functions:
        for b in f.blocks:
            out, changed = [], False
            for inst in b.instructions:
                si = inst.sync_info
                if si is not None and si.on_wait and len(si.on_wait) > 1:
                    waits = list(si.on_wait)
                    for w in waits[:-1]:
                        nop = bass_rust.InstNoOp(
                            name=f"I-splitw-{k}", ins=[], outs=[]
                        )
                        k += 1
                        nop.engine = inst.engine
                        nop.sync_info = mybir.SyncInfo(on_wait=[w], on_update=[])
                        out.append(nop)
                    inst.sync_info = mybir.SyncInfo(
                        on_wait=[waits[-1]], on_update=list(si.on_update)
                    )
                    changed = True
                out.append(inst)
            if changed:
                b.instructions = out


def _chunks(C):
    out, c0 = [], 0
    while c0 < C:
        cw = min(CHUNK, C - c0)
        out.append((c0, cw))
        c0 += cw
    return out


def _build_nc(C, use_b2, repeat=1):
    nc = bass.Bass()
    xT = nc.declare_dram_parameter("xT", [D_MODEL, C], F32R, isOutput=False)
    w1 = nc.declare_dram_parameter("w1", [D_MODEL, D_FF], F32R, isOutput=False)
    w2 = nc.declare_dram_parameter("w2", [D_FF, D_MODEL], F32R, isOutput=False)
    wtb = nc.declare_dram_parameter("wtb", [P, C], F32, isOutput=False)
    b1c = nc.declare_dram_parameter("b1c", [P, FO], F32, isOutput=False)
    b2c = nc.declare_dram_parameter("b2c", [P, DO], F32, isOutput=False)
    yT = nc.declare_dram_parameter("yT", [D_MODEL, C], F32, isOutput=True)

    xTr = xT.ap().rearrange("(ko p) c -> p ko c", p=P)
    yTr = yT.ap().rearrange("(do p) c -> p do c", p=P)
    w1r = w1.ap().rearrange("(ko p) f -> p ko f", p=P)
    w2r = w2.ap().rearrange("(fo p) d -> p fo d", p=P)

    relu = mybir.ActivationFunctionType.Relu

    with TileContext(nc) as tc:
        with (
            tc.tile_pool(name="wpool", bufs=1) as wpool,
            tc.tile_pool(name="xpool", bufs=2) as xpool,
            tc.tile_pool(name="hpool", bufs=1) as hpool,
            tc.tile_pool(name="ypool", bufs=3) as ypool,
            tc.tile_pool(name="ps1", bufs=3, space="PSUM") as pspool1,
            tc.tile_pool(name="ps2", bufs=3, space="PSUM") as pspool2,
        ):
            # resident weights, one tile per 128-row slice so matmuls can
            # start as soon as their slice lands
            w1s = [wpool.tile([P, D_FF], F32R, tag=f"w1_{ko}", name=f"w1_{ko}") for ko in range(KO)]
            for ko in range(KO):
                nc.sync.dma_start(w1s[ko][:], w1r[:, ko, :])
            w2s = [wpool.tile([P, D_MODEL], F32R, tag=f"w2_{fo}", name=f"w2_{fo}") for fo in range(FO)]
            for fo in range(FO):
                nc.sync.dma_start(w2s[fo][:], w2r[:, fo, :])
            wts = wpool.tile([P, C], F32, tag="wts", name="wts")
            nc.sync.dma_start(wts[:], wtb.ap())
            b1s = wpool.tile([P, FO], F32, tag="b1s", name="b1s")
            nc.sync.dma_start(b1s[:], b1c.ap())
            b2s = wpool.tile([P, DO], F32, tag="b2s", name="b2s")
            nc.sync.dma_start(b2s[:], b2c.ap())

            def body():
              for c0, cw in _chunks(C):
                xt = xpool.tile([P, KO, CHUNK], F32R, tag="xt", name="xt")[:, :, :cw]
                nc.sync.dma_start(xt, xTr[:, :, c0 : c0 + cw])
                ht = hpool.tile([P, FO, CHUNK], F32R, tag="ht", name="ht")[:, :, :cw]
                for fo in range(FO):
                    ps = pspool1.tile([P, CHUNK], F32, tag="ps1", name="ps1")[:, :cw]
                    for ko in range(KO):
                        nc.tensor.matmul(
                            ps,
                            w1s[ko][:, fo * P : (fo + 1) * P],
                            xt[:, ko, :],
                            start=(ko == 0),
                            stop=(ko == KO - 1),
                        )
                    nc.scalar.activation(
                        ht[:, fo, :], ps, relu, bias=b1s[:, fo : fo + 1]
                    )
                for do in range(DO):
                    ps2 = pspool2.tile([P, CHUNK], F32, tag="ps2", name="ps2")[:, :cw]
                    for fo in range(FO):
                        nc.tensor.matmul(
                            ps2,
                            w2s[fo][:, do * P : (do + 1) * P],
                            ht[:, fo, :],
                            start=(fo == 0),
                            stop=(fo == FO - 1),
                        )
                    yt = ypool.tile([P, CHUNK], F32, tag="yt", name="yt")[:, :cw]
                    if use_b2:
                        nc.vector.tensor_scalar_add(yt, ps2, b2s[:, do : do + 1])
                        nc.vector.tensor_mul(yt, yt, wts[:, c0 : c0 + cw])
                    else:
                        nc.vector.tensor_mul(yt, ps2, wts[:, c0 : c0 + cw])
                    nc.sync.dma_start(yTr[:, do, c0 : c0 + cw], yt)

            if repeat > 1:
                # hardware loop around the steady-state pass, used only for
                # benchmarking (delta-timing across repeat counts)
                with tc.For_i(0, repeat, 1):
                    body()
            else:
                body()

    _split_multiwait(nc)
    return nc


def kernel(x, gate_w, gate_b, w1, b1, w2, b2, top_k):
    x = np.asarray(x, np.float32)
    gate_w = np.asarray(gate_w, np.float32)
    gate_b = np.asarray(gate_b, np.float32)
    w1 = np.ascontiguousarray(np.asarray(w1, np.float32))
    b1 = np.asarray(b1, np.float32)
    w2 = np.ascontiguousarray(np.asarray(w2, np.float32))
    b2 = np.asarray(b2, np.float32)
    k = int(top_k)

    B, S, D = x.shape
    E = gate_w.shape[-1]
    T = B * S
    xf = np.ascontiguousarray(x.reshape(T, D))

    # --- host routing (the all-to-all shard step) ---
    # fp64 gate for tie-stable top-k: verified to match fp32 jax top_k
    logits64 = xf.astype(np.float64) @ gate_w.astype(np.float64) + gate_b
    order = np.argsort(-logits64, axis=-1, kind="stable")
    topk = order[:, :k]  # [T, k]
    selected = np.zeros((T, E), bool)
    np.put_along_axis(selected, topk, True, axis=-1)
    sparse = np.where(selected, logits64, NEG_INF)
    m = sparse.max(axis=-1, keepdims=True)
    ew = np.exp(sparse - m)
    ew /= ew.sum(axis=-1, keepdims=True)  # [T, E]; exactly 0 off the top-k

    idx = [np.nonzero(selected[:, e])[0] for e in range(E)]
    cnts = [len(i) for i in idx]
    C = max(256, -(-max(cnts) // 256) * 256)

    # --- per-core shards ---
    in_maps = []
    for e in range(E):
        cnt = cnts[e]
        xT = np.zeros((D, C), np.float32)
        xT[:, :cnt] = xf[idx[e]].T
        wtb = np.zeros((P, C), np.float32)
        wtb[:, :cnt] = ew[idx[e], e].astype(np.float32)[None, :]
        in_maps.append(
            {
                "xT": xT,
                "w1": w1[e],
                "w2": w2[e],
                "wtb": wtb,
                "b1c": np.ascontiguousarray(b1[e].reshape(FO, P).T),
                "b2c": np.ascontiguousarray(b2[e].reshape(DO, P).T),
            }
        )

    use_b2 = bool(np.any(b2))
    key = (C, use_b2)
    if key not in _nc_cache:
        _nc_cache[key] = _build_nc(C, use_b2)
    nc = _nc_cache[key]

    res = run_bass_kernel_spmd(nc, in_maps, list(range(N_CORES)))

    # --- unshard: scatter-add weighted expert outputs ---
    out = np.zeros((T, D), np.float32)
    for e in range(E):
        yT = res.results[e]["yT"]  # [D, C]
        out[idx[e]] += yT[:, : cnts[e]].T
    return out.reshape(B, S, D)
